# revision 16
# baseline (speedup 1.0000x reference)
"""Trainium2 Bass kernel for the minGRU problem.

Problem: hidden [8, 8192, 512] fp32, Ws [2, 1536, 512] fp32 (two stacked
minGRU layers with highway gates). Output [8, 8192, 512] fp32.

Math (per layer, linear-space equivalent of the reference's log-space scan):
    proj = hidden @ W.T                    # [T, 3H]
    z = sigmoid(gate);  a = 1 - z
    g = max(inner + 0.5, sigmoid(inner))   # == where(inner>=0, inner+0.5, sigmoid(inner))
    b = z * g
    o_t = a_t * o_{t-1} + b_t              # first-order scan along T (fp32 state)
    w = sigmoid(highway)
    hidden' = hidden + w * (o - hidden)

Sharding: data-parallel, one batch sample per NeuronCore (8 cores).

On-chip layout is [channel, time]: the projection matmul runs with W^T as
the stationary operand (contract over h on partitions) and hidden^T as the
moving operand, producing proj^T tiles [d, t] in PSUM; the scan runs along
the free (time) dimension with the native DVE tensor_tensor_scan
instruction. Input hidden is transposed on load with the DMA XBAR
(2-byte dtype), weights are transposed/cast on host (tiny). Output is
transposed back with PE identity-transposes and stored fp32.

Numerics: fp16 for matmul inputs and elementwise planes (fp16 has 10
mantissa bits; all values here are O(10) so no range issues), fp32 for the
scan state/carries and PSUM accumulation.
"""

import sys

sys.path.insert(0, "/opt/trn_rl_repo")

from contextlib import ExitStack

import numpy as np

import concourse.bass as bass
import concourse.tile as tile
from concourse import masks, mybir
from concourse.bass_utils import run_bass_kernel_spmd

F16 = mybir.dt.float16
F32 = mybir.dt.float32
OP = mybir.AluOpType
AF = mybir.ActivationFunctionType

B, T, H, L = 8, 8192, 512, 2
D3 = 3 * H          # 1536
NH = H // 128       # 4  channel partition-tiles
ND = D3 // 128      # 12 projection partition-tiles
TC = 512            # time-chunk (PSUM bank free size in fp32)
NCORES = 8


# Pool (gpsimd) excluded: it has 8 Q7 cores and same-engine waits are real.
_ENG_NAME = {
    mybir.EngineType.PE: "PE",
    mybir.EngineType.Activation: "Activation",
    mybir.EngineType.DVE: "DVE",
    mybir.EngineType.SP: "SP",
}


def _strip_self_waits(nc):
    """Drop on_wait entries on an instruction that wait on its OWN engine's
    semaphore. Engines execute their stream in order, and engine semaphores
    are only incremented by that engine's instructions at completion, so a
    same-engine wait is always already satisfied. Walrus (core_v3 codegen)
    only allows one sync-wait command on some instruction encodings, and
    these redundant self-waits push instructions over the limit."""
    import re

    for fn in nc.m.functions:
        for blk in fn.blocks:
            for inst in blk.instructions:
                si = inst.sync_info
                eng = _ENG_NAME.get(getattr(inst, "engine", None))
                if si is None or eng is None or not si.on_wait:
                    continue
                pat = re.compile(rf"^{eng}_\d+$")
                kept = [w for w in si.on_wait if not (
                    w.sync_type == "semaphore" and pat.match(w.ant_name or ""))]
                if len(kept) != len(si.on_wait):
                    inst.sync_info = mybir.SyncInfo(
                        on_wait=kept, on_update=list(si.on_update)
                    )


def _split_multi_waits(nc):
    """Walrus's core_v3 codegen allows only ONE sync-wait command on most
    instruction encodings (MM/TT/ACT/...). Tile sometimes emits 2+. Split the
    extras onto NoOp instructions inserted just before, on the same engine —
    semantically identical (the engine is in-order; waits execute at the same
    program point)."""
    keep_types = ("InstEventSemaphore", "InstNoOp")
    ctr = [0]
    for fn in nc.m.functions:
        for blk in fn.blocks:
            insts = blk.instructions
            out = []
            changed = False
            for inst in insts:
                si = inst.sync_info
                if (
                    si is not None
                    and len(si.on_wait) > 1
                    and type(inst).__name__ not in keep_types
                ):
                    for w in si.on_wait[:-1]:
                        ctr[0] += 1
                        out.append(
                            mybir.InstNoOp(
                                name=f"WSPLIT-{ctr[0]}",
                                ins=[],
                                outs=[],
                                engine=inst.engine,
                                sync_info=mybir.SyncInfo(on_wait=[w], on_update=[]),
                            )
                        )
                    inst.sync_info = mybir.SyncInfo(
                        on_wait=[si.on_wait[-1]], on_update=list(si.on_update)
                    )
                    changed = True
                out.append(inst)
            if changed:
                blk.instructions = out


def build_nc(t=T, scan_on_gpsimd=0, split_waits=True, kg=2, gp_ops=""):
    """kg: k-group size (stationary-weight reuse across kg chunks).
    gp_ops: subset of "deh" — which mix ops run on gpsimd instead of DVE
    ("d": o-h, "e": d*w, "h": h'=e+h)."""
    nk = t // TC
    assert nk % kg == 0
    nc = bass.Bass()
    hid_d = nc.declare_dram_parameter("hidden_f16", [t, H], F16, isOutput=False)
    wt_d = nc.declare_dram_parameter("wt_f16", [L, NH, 128, D3], F16, isOutput=False)
    out_d = nc.declare_dram_parameter("out_f32", [t, H], F32, isOutput=True)

    def eng(op):
        return nc.gpsimd if op in gp_ops else nc.vector

    with ExitStack() as ctx:
        tc_ = ctx.enter_context(tile.TileContext(nc))
        consts = ctx.enter_context(tc_.tile_pool(name="consts", bufs=1))
        h0p = ctx.enter_context(tc_.tile_pool(name="h0", bufs=6))
        h1p = ctx.enter_context(tc_.tile_pool(name="h1", bufs=nk))
        work = ctx.enter_context(tc_.tile_pool(name="work", bufs=2))
        scanp = ctx.enter_context(tc_.tile_pool(name="scan", bufs=3))
        outp = ctx.enter_context(tc_.tile_pool(name="outp", bufs=2))
        psum = ctx.enter_context(tc_.tile_pool(name="psum", bufs=6, space="PSUM"))
        psumtr = ctx.enter_context(tc_.tile_pool(name="psumtr", bufs=2, space="PSUM"))

        ident = consts.tile([128, 128], F16)
        masks.make_identity(nc, ident[:])

        # weights: per layer, [128 h-part, NH h-tile, D3] fp16 (lhsT layout)
        wt_sb = []
        for l in range(L):
            w = consts.tile([128, NH, D3], F16, tag=f"wt{l}")
            nc.sync.dma_start(out=w[:], in_=wt_d[l].rearrange("n p d -> p n d"))
            wt_sb.append(w)

        # layer-1 output plane, held as per-chunk tiles for fine-grained deps
        h1_tiles = [None] * nk

        prev_o = None
        for l in range(L):
            for k0 in range(0, nk, kg):
                hins = []
                for kk in range(kg):
                    k = k0 + kk
                    if l == 0:
                        # DMA XBAR transpose: out[p, hc, t] = hid[t, hc*128+p]
                        hin = h0p.tile([128, NH, TC], F16, tag="h0")
                        nc.sync.dma_start_transpose(
                            out=hin[:], in_=hid_d[k * TC : (k + 1) * TC, :]
                        )
                        hins.append(hin)
                    else:
                        hins.append(h1_tiles[k])

                # ---- projection matmuls; lhsT stationary across the k-group
                pss = [[] for _ in range(kg)]
                for dc in range(ND):
                    pts = [
                        psum.tile([128, TC], F32, tag="mm", name=f"pt{kk}")
                        for kk in range(kg)
                    ]
                    for hc in range(NH):
                        for kk in range(kg):
                            nc.tensor.matmul(
                                pts[kk][:],
                                wt_sb[l][:, hc, dc * 128 : (dc + 1) * 128],
                                hins[kk][:, hc, :],
                                start=(hc == 0),
                                stop=(hc == NH - 1),
                            )
                    for kk in range(kg):
                        pss[kk].append(pts[kk])

                for kk in range(kg):
                    k = k0 + kk
                    ps = pss[kk]
                    hin = hins[kk]
                    # ---- activations (ScalarE) ----
                    z = work.tile([128, NH, TC], F16, tag="z")
                    s = work.tile([128, NH, TC], F16, tag="s")
                    w_ = work.tile([128, NH, TC], F16, tag="w")
                    b = work.tile([128, NH, TC], F16, tag="b")
                    for c in range(NH):
                        nc.scalar.activation(s[:, c, :], ps[c][:], AF.Sigmoid)
                        nc.scalar.activation(z[:, c, :], ps[NH + c][:], AF.Sigmoid)
                        nc.scalar.activation(w_[:, c, :], ps[2 * NH + c][:], AF.Sigmoid)

                    # ---- g = (inner + 0.5) max s  (in-place into s) ----
                    for c in range(NH):
                        nc.vector.scalar_tensor_tensor(
                            out=s[:, c, :],
                            in0=ps[c][:],
                            scalar=0.5,
                            in1=s[:, c, :],
                            op0=OP.add,
                            op1=OP.max,
                        )
                    # b = z * g; a = 1 - z (in-place into z)
                    for c in range(NH):
                        nc.vector.tensor_tensor(
                            b[:, c, :], z[:, c, :], s[:, c, :], OP.mult
                        )
                        nc.vector.tensor_scalar(
                            z[:, c, :], z[:, c, :], -1.0, 1.0, OP.mult, OP.add
                        )

                    # ---- scan: o_t = a*o + b (fp32 state), chained over chunks
                    o = scanp.tile([128, NH, TC], F32, tag="o")
                    for c in range(NH):
                        init = 0.0 if k == 0 else prev_o[:, c, TC - 1 : TC]
                        se = nc.gpsimd if c < scan_on_gpsimd else nc.vector
                        se.tensor_tensor_scan(
                            o[:, c, :], z[:, c, :], b[:, c, :], init, OP.mult, OP.add
                        )
                    prev_o = o

                    # ---- highway mix: h' = h + w*(o - h) ----
                    if l == 0:
                        ho = h1p.tile([128, NH, TC], F16, tag="h1")
                        h1_tiles[k] = ho
                    else:
                        ho = work.tile([128, NH, TC], F16, tag="ho")
                    for c in range(NH):
                        # d = o - h (into b's tile, already consumed by the scan)
                        eng("d").tensor_tensor(
                            b[:, c, :], o[:, c, :], hin[:, c, :], OP.subtract
                        )
                        # e = d * w (in-place into w_)
                        eng("e").tensor_tensor(
                            w_[:, c, :], b[:, c, :], w_[:, c, :], OP.mult
                        )
                        eng("h").tensor_tensor(
                            ho[:, c, :], w_[:, c, :], hin[:, c, :], OP.add
                        )
                    if l == 1:
                        # ---- output transpose (PE) + upcast copy + store ----
                        ot = outp.tile([128, TC // 128, H], F32, tag="ot")
                        for tk in range(TC // 128):
                            ptr = psumtr.tile([128, H], F16, tag="tr")
                            for c in range(NH):
                                nc.tensor.transpose(
                                    ptr[:, c * 128 : (c + 1) * 128],
                                    ho[:, c, tk * 128 : (tk + 1) * 128],
                                    ident[:],
                                )
                            nc.scalar.copy(ot[:, tk, :], ptr[:])
                        nc.sync.dma_start(
                            out=out_d[k * TC : (k + 1) * TC, :].rearrange(
                                "(tk p) h -> p tk h", p=128
                            ),
                            in_=ot[:],
                        )
            if l == 0:
                prev_o = None
    if split_waits:
        _split_multi_waits(nc)
    return nc


_NC_CACHE = {}


def _get_nc(t=T, scan_on_gpsimd=0):
    key = (t, scan_on_gpsimd)
    if key not in _NC_CACHE:
        _NC_CACHE[key] = build_nc(t, scan_on_gpsimd)
    return _NC_CACHE[key]


def _prep_inputs(hidden, Ws):
    h16 = np.ascontiguousarray(hidden).astype(np.float16)
    wt = np.ascontiguousarray(np.transpose(Ws, (0, 2, 1)))  # [L, H, D3]
    wt = wt.reshape(L, NH, 128, D3).astype(np.float16)
    return h16, wt


def kernel(hidden, Ws):
    assert hidden.shape == (B, T, H) and Ws.shape == (L, D3, H)
    h16, wt = _prep_inputs(hidden, Ws)
    nc = _get_nc()
    in_maps = [{"hidden_f16": h16[i], "wt_f16": wt} for i in range(NCORES)]
    res = run_bass_kernel_spmd(nc, in_maps, list(range(NCORES)))
    out = np.stack([res.results[i]["out_f32"] for i in range(NCORES)])
    return out.astype(np.float32)


# revision 19
# speedup vs baseline: 1.2163x; 1.2163x over previous
"""Trainium2 Bass kernel for the minGRU problem.

Problem: hidden [8, 8192, 512] fp32, Ws [2, 1536, 512] fp32 (two stacked
minGRU layers with highway gates). Output [8, 8192, 512] fp32.

Math (per layer, linear-space equivalent of the reference's log-space scan):
    proj = hidden @ W.T                    # [T, 3H]
    z = sigmoid(gate);  a = 1 - z
    g = max(inner + 0.5, sigmoid(inner))   # == where(inner>=0, inner+0.5, sigmoid(inner))
    b = z * g
    o_t = a_t * o_{t-1} + b_t              # first-order scan along T (fp32 state)
    w = sigmoid(highway)
    hidden' = hidden + w * (o - hidden)

Sharding: data-parallel, one batch sample per NeuronCore (8 cores).

On-chip layout is [channel, time]: the projection matmul runs with W^T as
the stationary operand (contract over h on partitions) and hidden^T as the
moving operand, producing proj^T tiles [d, t] in PSUM; the scan runs along
the free (time) dimension with the native DVE tensor_tensor_scan
instruction. Input hidden is transposed on load with the DMA XBAR
(2-byte dtype), weights are transposed/cast on host (tiny). Output is
transposed back with PE identity-transposes and stored fp32.

Numerics: fp16 for matmul inputs and elementwise planes (fp16 has 10
mantissa bits; all values here are O(10) so no range issues), fp32 for the
scan state/carries and PSUM accumulation.
"""

import sys

sys.path.insert(0, "/opt/trn_rl_repo")

from contextlib import ExitStack

import numpy as np

import concourse.bass as bass
import concourse.tile as tile
from concourse import masks, mybir
from concourse.bass_utils import run_bass_kernel_spmd

F16 = mybir.dt.float16
F32 = mybir.dt.float32
OP = mybir.AluOpType
AF = mybir.ActivationFunctionType

B, T, H, L = 8, 8192, 512, 2
D3 = 3 * H          # 1536
NH = H // 128       # 4  channel partition-tiles
ND = D3 // 128      # 12 projection partition-tiles
TC = 512            # time-chunk (PSUM bank free size in fp32)
NCORES = 8


# Pool (gpsimd) excluded: it has 8 Q7 cores and same-engine waits are real.
_ENG_NAME = {
    mybir.EngineType.PE: "PE",
    mybir.EngineType.Activation: "Activation",
    mybir.EngineType.DVE: "DVE",
    mybir.EngineType.SP: "SP",
}


def _strip_self_waits(nc):
    """Drop on_wait entries on an instruction that wait on its OWN engine's
    semaphore. Engines execute their stream in order, and engine semaphores
    are only incremented by that engine's instructions at completion, so a
    same-engine wait is always already satisfied. Walrus (core_v3 codegen)
    only allows one sync-wait command on some instruction encodings, and
    these redundant self-waits push instructions over the limit."""
    import re

    for fn in nc.m.functions:
        for blk in fn.blocks:
            for inst in blk.instructions:
                si = inst.sync_info
                eng = _ENG_NAME.get(getattr(inst, "engine", None))
                if si is None or eng is None or not si.on_wait:
                    continue
                pat = re.compile(rf"^{eng}_\d+$")
                kept = [w for w in si.on_wait if not (
                    w.sync_type == "semaphore" and pat.match(w.ant_name or ""))]
                if len(kept) != len(si.on_wait):
                    inst.sync_info = mybir.SyncInfo(
                        on_wait=kept, on_update=list(si.on_update)
                    )


def _split_multi_waits(nc):
    """Walrus's core_v3 codegen allows only ONE sync-wait command on most
    instruction encodings (MM/TT/ACT/...). Tile sometimes emits 2+. Split the
    extras onto NoOp instructions inserted just before, on the same engine —
    semantically identical (the engine is in-order; waits execute at the same
    program point)."""
    keep_types = ("InstEventSemaphore", "InstNoOp")
    ctr = [0]
    for fn in nc.m.functions:
        for blk in fn.blocks:
            insts = blk.instructions
            out = []
            changed = False
            for inst in insts:
                si = inst.sync_info
                if (
                    si is not None
                    and len(si.on_wait) > 1
                    and type(inst).__name__ not in keep_types
                ):
                    for w in si.on_wait[:-1]:
                        ctr[0] += 1
                        out.append(
                            mybir.InstNoOp(
                                name=f"WSPLIT-{ctr[0]}",
                                ins=[],
                                outs=[],
                                engine=inst.engine,
                                sync_info=mybir.SyncInfo(on_wait=[w], on_update=[]),
                            )
                        )
                    inst.sync_info = mybir.SyncInfo(
                        on_wait=[si.on_wait[-1]], on_update=list(si.on_update)
                    )
                    changed = True
                out.append(inst)
            if changed:
                blk.instructions = out


def build_nc(
    t=T,
    scan_on_gpsimd=0,
    split_waits=True,
    kg=2,
    gp_ops="",
    scan_f16=True,
    a_on_act=True,
):
    """kg: k-group size (stationary-weight reuse across kg chunks).
    gp_ops: subset of "deh" — which mix ops run on gpsimd instead of DVE
    ("d": o-h, "e": d*w, "h": h'=e+h).
    scan_f16: scan output (and carry) in fp16 -> d runs in the DVE 2x mode.
    a_on_act: a = sigmoid(-gate) on ScalarE instead of 1-z on DVE."""
    nk = t // TC
    assert nk % kg == 0
    nc = bass.Bass()
    hid_d = nc.declare_dram_parameter("hidden_f16", [t, H], F16, isOutput=False)
    wt_d = nc.declare_dram_parameter("wt_f16", [L, NH, 128, D3], F16, isOutput=False)
    out_d = nc.declare_dram_parameter("out_f32", [t, H], F32, isOutput=True)

    def eng(op):
        return nc.gpsimd if op in gp_ops else nc.vector

    with ExitStack() as ctx:
        tc_ = ctx.enter_context(tile.TileContext(nc))
        consts = ctx.enter_context(tc_.tile_pool(name="consts", bufs=1))
        h0p = ctx.enter_context(tc_.tile_pool(name="h0", bufs=6))
        h1p = ctx.enter_context(tc_.tile_pool(name="h1", bufs=nk))
        work = ctx.enter_context(tc_.tile_pool(name="work", bufs=2))
        scanp = ctx.enter_context(tc_.tile_pool(name="scan", bufs=3))
        outp = ctx.enter_context(tc_.tile_pool(name="outp", bufs=2))
        psum = ctx.enter_context(tc_.tile_pool(name="psum", bufs=6, space="PSUM"))
        psumtr = ctx.enter_context(tc_.tile_pool(name="psumtr", bufs=2, space="PSUM"))

        ident = consts.tile([128, 128], F16)
        masks.make_identity(nc, ident[:])

        # weights: per layer, [128 h-part, NH h-tile, D3] fp16 (lhsT layout)
        wt_sb = []
        for l in range(L):
            w = consts.tile([128, NH, D3], F16, tag=f"wt{l}")
            nc.sync.dma_start(out=w[:], in_=wt_d[l].rearrange("n p d -> p n d"))
            wt_sb.append(w)

        # layer-1 output plane, held as per-chunk tiles for fine-grained deps
        h1_tiles = [None] * nk

        prev_o = None
        for l in range(L):
            for k0 in range(0, nk, kg):
                hins = []
                for kk in range(kg):
                    k = k0 + kk
                    if l == 0:
                        # DMA XBAR transpose: out[p, hc, t] = hid[t, hc*128+p]
                        hin = h0p.tile([128, NH, TC], F16, tag="h0")
                        nc.sync.dma_start_transpose(
                            out=hin[:], in_=hid_d[k * TC : (k + 1) * TC, :]
                        )
                        hins.append(hin)
                    else:
                        hins.append(h1_tiles[k])

                # ---- projection matmuls; lhsT stationary across the k-group
                pss = [[] for _ in range(kg)]
                for dc in range(ND):
                    pts = [
                        psum.tile([128, TC], F32, tag="mm", name=f"pt{kk}")
                        for kk in range(kg)
                    ]
                    for hc in range(NH):
                        for kk in range(kg):
                            nc.tensor.matmul(
                                pts[kk][:],
                                wt_sb[l][:, hc, dc * 128 : (dc + 1) * 128],
                                hins[kk][:, hc, :],
                                start=(hc == 0),
                                stop=(hc == NH - 1),
                            )
                    for kk in range(kg):
                        pss[kk].append(pts[kk])

                for kk in range(kg):
                    k = k0 + kk
                    ps = pss[kk]
                    hin = hins[kk]
                    # ---- activations (ScalarE) ----
                    z = work.tile([128, NH, TC], F16, tag="z")
                    s = work.tile([128, NH, TC], F16, tag="s")
                    w_ = work.tile([128, NH, TC], F16, tag="w")
                    b = work.tile([128, NH, TC], F16, tag="b")
                    if a_on_act:
                        a = work.tile([128, NH, TC], F16, tag="a")
                    for c in range(NH):
                        nc.scalar.activation(s[:, c, :], ps[c][:], AF.Sigmoid)
                        nc.scalar.activation(z[:, c, :], ps[NH + c][:], AF.Sigmoid)
                        nc.scalar.activation(w_[:, c, :], ps[2 * NH + c][:], AF.Sigmoid)
                        if a_on_act:
                            nc.scalar.activation(
                                a[:, c, :], ps[NH + c][:], AF.Sigmoid, scale=-1.0
                            )

                    # ---- g = (inner + 0.5) max s  (in-place into s) ----
                    for c in range(NH):
                        nc.vector.scalar_tensor_tensor(
                            out=s[:, c, :],
                            in0=ps[c][:],
                            scalar=0.5,
                            in1=s[:, c, :],
                            op0=OP.add,
                            op1=OP.max,
                        )
                    # b = z * g; a = 1 - z (in-place into z) unless ACT did it
                    for c in range(NH):
                        nc.vector.tensor_tensor(
                            b[:, c, :], z[:, c, :], s[:, c, :], OP.mult
                        )
                        if not a_on_act:
                            nc.vector.tensor_scalar(
                                z[:, c, :], z[:, c, :], -1.0, 1.0, OP.mult, OP.add
                            )
                    if not a_on_act:
                        a = z

                    # ---- scan: o_t = a*o + b (fp32 state), chained over chunks
                    o = scanp.tile([128, NH, TC], F16 if scan_f16 else F32, tag="o")
                    for c in range(NH):
                        init = 0.0 if k == 0 else prev_o[:, c, TC - 1 : TC]
                        se = nc.gpsimd if c < scan_on_gpsimd else nc.vector
                        se.tensor_tensor_scan(
                            o[:, c, :], a[:, c, :], b[:, c, :], init, OP.mult, OP.add
                        )
                    prev_o = o

                    # ---- highway mix: h' = h + w*(o - h) ----
                    if l == 0:
                        ho = h1p.tile([128, NH, TC], F16, tag="h1")
                        h1_tiles[k] = ho
                    else:
                        ho = work.tile([128, NH, TC], F16, tag="ho")
                    for c in range(NH):
                        # d = o - h (into b's tile, already consumed by the scan)
                        eng("d").tensor_tensor(
                            b[:, c, :], o[:, c, :], hin[:, c, :], OP.subtract
                        )
                        # e = d * w (in-place into w_)
                        eng("e").tensor_tensor(
                            w_[:, c, :], b[:, c, :], w_[:, c, :], OP.mult
                        )
                        eng("h").tensor_tensor(
                            ho[:, c, :], w_[:, c, :], hin[:, c, :], OP.add
                        )
                    if l == 1:
                        # ---- output transpose (PE) + upcast copy + store ----
                        ot = outp.tile([128, TC // 128, H], F32, tag="ot")
                        for tk in range(TC // 128):
                            ptr = psumtr.tile([128, H], F16, tag="tr")
                            for c in range(NH):
                                nc.tensor.transpose(
                                    ptr[:, c * 128 : (c + 1) * 128],
                                    ho[:, c, tk * 128 : (tk + 1) * 128],
                                    ident[:],
                                )
                            nc.scalar.copy(ot[:, tk, :], ptr[:])
                        nc.sync.dma_start(
                            out=out_d[k * TC : (k + 1) * TC, :].rearrange(
                                "(tk p) h -> p tk h", p=128
                            ),
                            in_=ot[:],
                        )
            if l == 0:
                prev_o = None
    if split_waits:
        _split_multi_waits(nc)
    return nc


_NC_CACHE = {}


def _get_nc(t=T, scan_on_gpsimd=0):
    key = (t, scan_on_gpsimd)
    if key not in _NC_CACHE:
        _NC_CACHE[key] = build_nc(t, scan_on_gpsimd)
    return _NC_CACHE[key]


def _prep_inputs(hidden, Ws):
    h16 = np.ascontiguousarray(hidden).astype(np.float16)
    wt = np.ascontiguousarray(np.transpose(Ws, (0, 2, 1)))  # [L, H, D3]
    wt = wt.reshape(L, NH, 128, D3).astype(np.float16)
    return h16, wt


def kernel(hidden, Ws):
    assert hidden.shape == (B, T, H) and Ws.shape == (L, D3, H)
    h16, wt = _prep_inputs(hidden, Ws)
    nc = _get_nc()
    in_maps = [{"hidden_f16": h16[i], "wt_f16": wt} for i in range(NCORES)]
    res = run_bass_kernel_spmd(nc, in_maps, list(range(NCORES)))
    out = np.stack([res.results[i]["out_f32"] for i in range(NCORES)])
    return out.astype(np.float32)
